# revision 18
# baseline (speedup 1.0000x reference)
"""Trainium2 Bass kernel for nn_MultiHeadAttention (B=2, S=2048, E=1024, H=16).

Sharding: 8 NeuronCores = data-parallel over the 2 batches x tensor-parallel
over the 16 heads in 4 groups of 4 heads (Wq/Wk/Wv split column-wise, Wo
row-wise).  Each core computes a full-[S, E] partial of its batch's output;
the host sums the 4 head-group partials per batch.

Per-core pipeline (ACT exp stream is the pace-setter at ~133us):
  Q.T/K.T[n, s]  per 512-col window: psX-accumulated e-outer matmuls
                 chasing column-block DMAs (first exp at ~11us).
  S.T_h[k, q]    [64,128]x[64,512] per (window, head-pair, k-chunk),
                 head pair packed in one [128,1024] PSUM tile.
  P.T            one ACT exp op per (window, pair, k-chunk); fp16 out.
  O[q, d|sum]    TRANSPOSED P.V: stationary = P.T chunk [128k,128q],
                 moving = [V_h | ones] [128k, 65] -> accumulates
                 [128q, 65] in sub-bank PSUM slices (2x fewer PE rows
                 than the O.T orientation).  Lagged one (window,pair)
                 behind the exp stream.
  normalize      DVE reciprocal of col 64 + per-partition scalar mult.
  O.T            crossbar DMA transpose [128q,128hd] -> [128hd,128q].
  out[m, :]      stationary oT chunk [128 hd, 128 m] x moving Wo.T
                 [128 hd, 512 e], 2-chunk contraction, DVE copy + SWDGE
                 store.

dtypes: all matmul operands fp16 (1 cyc/row on PE); PSUM accumulation fp32;
softmax exp on fp32 scores.
"""

import numpy as np
from contextlib import ExitStack

import concourse.bass as bass
import concourse.mybir as mybir
import concourse.tile as tile
from concourse.tile import ScopedClock
from concourse.bass_utils import run_bass_kernel_spmd

# ---------------------------------------------------------------------------
# Workarounds for the walrus build on this stack, which rejects more than ONE
# semaphore wait per instruction ("Too many sync wait commands").
# ---------------------------------------------------------------------------
_orig_commit_instruction = tile.TileContext._commit_instruction


def _commit_instruction(self, inst, lazy_reg_writes=True):
    si = getattr(inst, "sync_info", None)
    if si is not None and si.on_wait and len(si.on_wait) > 1:
        waits = list(si.on_wait)
        for w in waits[:-1]:
            nop = mybir.InstNoOp(
                name=self.nc.get_next_instruction_name(),
                ins=[], outs=[], engine=inst.engine,
            )
            nop.bass_nofuse = True
            nop.sync_info = mybir.SyncInfo(on_wait=[w], on_update=[])
            _orig_commit_instruction(self, nop, lazy_reg_writes=False)
        inst.sync_info = mybir.SyncInfo(
            on_wait=[waits[-1]], on_update=list(si.on_update or [])
        )
    return _orig_commit_instruction(self, inst, lazy_reg_writes)


def _drain_and_barrier(self, tick_clock, wait_clock):
    nc = self.nc
    drain_inst = nc.sync.drain()
    wait_clock.add_sem_waits(
        drain_inst.ins, ScopedClock({None: tick_clock.global_clock})
    )
    si = drain_inst.ins.sync_info
    waits = list(si.on_wait) if si and si.on_wait else []
    if len(waits) > 1:
        drain_inst.ins.sync_info = mybir.SyncInfo(
            on_wait=waits[:1], on_update=list(si.on_update or [])
        )
        for w in waits[1:]:
            extra = nc.sync.drain()
            esi = extra.ins.sync_info
            extra.ins.sync_info = mybir.SyncInfo(
                on_wait=[w],
                on_update=list(esi.on_update or []) if esi else [],
            )
    nc.all_engine_barrier()
    assert self.sems is not None
    popped = nc._tile_sem_poison_stack.pop()
    assert popped is self._sem_poison
    nc.clear_and_free_semaphores(list(self.sems.allocated().values()))
    nc.all_engine_barrier()


def _apply_tilefix():
    tile.TileContext._commit_instruction = _commit_instruction
    tile.TileContext._drain_and_barrier = _drain_and_barrier


_apply_tilefix()

# ---------------------------------------------------------------------------
# Problem constants (hardcoded)
# ---------------------------------------------------------------------------
B, S, E, H = 2, 2048, 1024, 16
HC, D = 4, 64              # heads per core, head dim
NCORES = 8
NE = E // 128              # 8  e-chunks
NW = S // 512              # 4  q/k windows
NK = S // 128              # 16 k-chunks
NM = S // 128              # 16 m-chunks

F32 = mybir.dt.float32
FP16 = mybir.dt.float16


def build(ptbufs=18):
    nc = bass.Bass()
    xqT = nc.dram_tensor("xqT", [E, S], FP16, kind="ExternalInput")
    xkT = nc.dram_tensor("xkT", [E, S], FP16, kind="ExternalInput")
    xvT = nc.dram_tensor("xvT", [E, S], FP16, kind="ExternalInput")
    wqT = nc.dram_tensor("wqT", [E, 256], FP16, kind="ExternalInput")
    wkT = nc.dram_tensor("wkT", [E, 256], FP16, kind="ExternalInput")
    wvT = nc.dram_tensor("wvT", [E, 256], FP16, kind="ExternalInput")
    woT = nc.dram_tensor("woT", [256, E], FP16, kind="ExternalInput")
    out = nc.dram_tensor("out", [S, E], F32, kind="ExternalOutput")
    out16 = nc.dram_tensor("out16", [512, E], FP16, kind="ExternalOutput")
    identT = nc.dram_tensor("identT", [128, 128], FP16, kind="ExternalInput")

    with tile.TileContext(nc) as tc, ExitStack() as ctx:
        consts = ctx.enter_context(tc.tile_pool(name="consts", bufs=1))
        wpool = ctx.enter_context(tc.tile_pool(name="w", bufs=1))
        actpool = ctx.enter_context(tc.tile_pool(name="acts", bufs=1))
        xkpool = ctx.enter_context(tc.tile_pool(name="xk", bufs=3))
        xqpool = ctx.enter_context(tc.tile_pool(name="xq", bufs=3))
        xvpool = ctx.enter_context(tc.tile_pool(name="xv", bufs=3))
        ptpool = ctx.enter_context(tc.tile_pool(name="pT", bufs=ptbufs))
        onpool = ctx.enter_context(tc.tile_pool(name="on", bufs=4))
        rpool = ctx.enter_context(tc.tile_pool(name="recip", bufs=8))
        opool = ctx.enter_context(tc.tile_pool(name="outstage", bufs=2))
        psS = ctx.enter_context(tc.tile_pool(name="psS", bufs=2, space="PSUM"))
        psOV = ctx.enter_context(tc.tile_pool(name="psOV", bufs=1, space="PSUM"))
        psX = ctx.enter_context(tc.tile_pool(name="psX", bufs=2, space="PSUM"))

        # preload the exp table before the hot loop
        dummy = consts.tile([1, 8], F32)
        nc.vector.memset(dummy[:], 0.0)
        nc.scalar.activation(dummy[:], dummy[:], mybir.ActivationFunctionType.Exp)

        wq_sb = wpool.tile([128, NE, 256], FP16)
        wk_sb = wpool.tile([128, NE, 256], FP16)
        wv_sb = wpool.tile([128, NE, 256], FP16)
        wo_sb = wpool.tile([128, 2, E], FP16)

        qT_sb = actpool.tile([128, 2, S], FP16)        # [(2 heads x d), pair, s]
        kT_sb = actpool.tile([128, 2, S], FP16)
        v_sb = actpool.tile([128, NK, HC, 65], FP16)   # [s%128, k, h, V_h|ones]
        oT_sb = actpool.tile([128, 2, S], FP16, name="oT")  # [(h2 d), pair, s]

        nc.vector.memset(v_sb[:, :, :, 64:65], 1.0)

        # ---- DMA emission order on the SP queue (arrival order == need) ---
        def colblock(x, j):
            return x[:, j * 512:(j + 1) * 512].rearrange(
                "(ec p) s -> p ec s", p=128)

        def halfblock(x, j, h):
            return x[h * 512:(h + 1) * 512,
                     j * 512:(j + 1) * 512].rearrange(
                "(ec p) s -> p ec s", p=128)

        xq_blks = {}
        xk_blks = {}
        xv_blks = {}

        def load_x(pool, src, blks, j, tag, split=False):
            t = pool.tile([128, NE, 512], FP16, tag=tag, name=f"{tag}{j}")
            if split:
                # two half-e DMAs so the projection can chase the first half
                nc.sync.dma_start(t[:, 0:4, :], halfblock(src, j, 0))
                nc.sync.dma_start(t[:, 4:8, :], halfblock(src, j, 1))
            else:
                nc.sync.dma_start(t[:], colblock(src, j))
            blks[j] = t

        # arrival order == need order (single serialized DMA device)
        nc.sync.dma_start(wk_sb[:], wkT.rearrange("(ec p) n -> p ec n", p=128))
        load_x(xkpool, xkT, xk_blks, 0, "xkb", split=True)
        nc.sync.dma_start(wq_sb[:], wqT.rearrange("(ec p) n -> p ec n", p=128))
        load_x(xqpool, xqT, xq_blks, 0, "xqb", split=True)
        for j in range(1, NW):
            load_x(xkpool, xkT, xk_blks, j, "xkb", split=True)
        nc.sync.dma_start(wv_sb[:], wvT.rearrange("(ec p) n -> p ec n", p=128))
        load_x(xvpool, xvT, xv_blks, 0, "xvb")
        load_x(xvpool, xvT, xv_blks, 1, "xvb")
        load_x(xqpool, xqT, xq_blks, 1, "xqb")
        load_x(xvpool, xvT, xv_blks, 2, "xvb")
        load_x(xvpool, xvT, xv_blks, 3, "xvb")
        load_x(xqpool, xqT, xq_blks, 2, "xqb")
        load_x(xqpool, xqT, xq_blks, 3, "xqb")
        nc.sync.dma_start(wo_sb[:], woT.rearrange("(j p) e -> p j e", p=128))
        id_sb = wpool.tile([128, 128], FP16)
        nc.sync.dma_start(id_sb[:], identT[:, :])

        # ---- building blocks -------------------------------------------
        def wproj(w_sb, blk, dst, win):
            """project one 512-col window of x into dst[:, :, win] (2 psX)."""
            ws = slice(win * 512, (win + 1) * 512)
            for nch in range(2):
                ps = psX.tile([128, 512], F32, tag="px", name=f"pj{win}_{nch}")
                for e in range(NE):
                    nc.tensor.matmul(
                        ps[:],
                        w_sb[:, e, nch * 128:(nch + 1) * 128],
                        blk[:, e, :],
                        start=(e == 0), stop=(e == NE - 1))
                nc.vector.tensor_copy(dst[:, nch, ws], ps[:])

        def vproj(m):
            blk = xv_blks[m // 4]
            ps = psX.tile([128, 512], F32, tag="px", name=f"vp{m}")
            for e in range(NE):
                nc.tensor.matmul(
                    ps[:, 0:256],
                    blk[:, e, (m % 4) * 128:(m % 4 + 1) * 128],
                    wv_sb[:, e, :],
                    start=(e == 0), stop=(e == NE - 1))
            nc.vector.tensor_copy(
                v_sb[:, m, :, 0:64],
                ps[:, 0:256].rearrange("p (h c) -> p h c", h=HC))

        def ov_group(ovts, pair, pT, kc):
            """8 transposed-PV matmuls for one k-chunk; sub-bank psum accum."""
            for t_i, (ovt, qlocs) in enumerate(ovts):
                for si, (h2, ql) in enumerate(
                        [(h, q) for h in range(2) for q in qlocs]):
                    nc.tensor.matmul(
                        ovt[:, ql % 2, h2, :],
                        pT[:, h2 * 512 + ql * 128: h2 * 512 + (ql + 1) * 128],
                        v_sb[:, kc, 2 * pair + h2, :],
                        start=(kc == 0 and si == 0),
                        stop=(kc == NK - 1 and si == 3),
                        skip_group_check=True)

        def finalize(ovts, w, pair):
            """normalize + crossbar-transpose one (window, pair)."""
            for ovt, qlocs in ovts:
                for ql in qlocs:
                    o_n = onpool.tile([128, 128], FP16, tag="on")
                    for h2 in range(2):
                        rt = rpool.tile([128, 1], F32, tag="rt")
                        nc.vector.reciprocal(rt[:], ovt[:, ql % 2, h2, 64:65])
                        nc.vector.tensor_scalar_mul(
                            o_n[:, h2 * 64:(h2 + 1) * 64],
                            ovt[:, ql % 2, h2, 0:64],
                            rt[:])
                    qs = slice(w * 512 + ql * 128, w * 512 + (ql + 1) * 128)
                    nc.sync.dma_start_transpose(oT_sb[:, pair, qs], o_n[:])

        def outproj_half(m, j, stage):
            ps = psX.tile([128, 512], F32, tag="px", name=f"op{m}_{j}")
            for jp in range(2):
                nc.tensor.matmul(
                    ps[:],
                    oT_sb[:, jp, m * 128:(m + 1) * 128],
                    wo_sb[:, jp, j * 512:(j + 1) * 512],
                    start=(jp == 0), stop=(jp == 1))
            nc.vector.tensor_copy(stage[:, j * 512:(j + 1) * 512], ps[:])
            if j == 1:
                nc.gpsimd.dma_start(out[m * 128:(m + 1) * 128, :], stage[:])

        # ---- global-slot schedule --------------------------------------
        # slot g = p*16 + kc carries scores(p,kc)+exp; OV work is lagged
        # OVLAG slots behind the exp stream (rolling across p boundaries).
        OVLAG = 10
        TAILOV = 8            # OV groups left for the post-stream tail
        from collections import defaultdict
        extras_pre = defaultdict(list)    # g -> thunks (feeders: proj work)
        extras_post = defaultdict(list)   # g -> thunks (drains: outproj)

        def sched_wproj(g, w_sb, blks, dst, win, nch=None):
            for n in ((0, 1) if nch is None else (nch,)):
                extras_pre[g].append(
                    lambda n=n, win=win: wproj1(w_sb, blks[win], dst, win, n))

        def wproj1(w_sb, blk, dst, win, nch, use_act=False):
            ws = slice(win * 512, (win + 1) * 512)
            ps = psX.tile([128, 512], F32, tag="px", name=f"pj{win}_{nch}")
            for e in range(NE):
                nc.tensor.matmul(
                    ps[:],
                    w_sb[:, e, nch * 128:(nch + 1) * 128],
                    blk[:, e, :],
                    start=(e == 0), stop=(e == NE - 1))
            if use_act:
                nc.scalar.copy(dst[:, nch, ws], ps[:])
            else:
                nc.vector.tensor_copy(dst[:, nch, ws], ps[:])

        # K windows 1-3 early in p0 (chasing the xk block DMAs)
        sched_wproj(2, wk_sb, xk_blks, kT_sb, 1, nch=0)
        sched_wproj(3, wk_sb, xk_blks, kT_sb, 1, nch=1)
        sched_wproj(6, wk_sb, xk_blks, kT_sb, 2, nch=0)
        sched_wproj(7, wk_sb, xk_blks, kT_sb, 2, nch=1)
        sched_wproj(10, wk_sb, xk_blks, kT_sb, 3, nch=0)
        sched_wproj(11, wk_sb, xk_blks, kT_sb, 3, nch=1)
        # V tiles 2 slots ahead of their OV consumer (max arrival slack)
        for m in range(NM):
            extras_pre[m + 9].append(lambda m=m: vproj(m))
        # remaining Q windows, off the critical path, in slack windows
        sched_wproj(15, wq_sb, xq_blks, qT_sb, 1, nch=0)
        sched_wproj(17, wq_sb, xq_blks, qT_sb, 1, nch=1)
        sched_wproj(40, wq_sb, xq_blks, qT_sb, 2, nch=0)
        sched_wproj(56, wq_sb, xq_blks, qT_sb, 2, nch=1)
        sched_wproj(72, wq_sb, xq_blks, qT_sb, 3, nch=0)
        sched_wproj(88, wq_sb, xq_blks, qT_sb, 3, nch=1)

        # outproj window w' after both its finalizes: units u0..3 at
        # p(2w'+2) slots 12..15, u4..7 at p(2w'+3) slots 1,3,5,7
        stages = {}

        def outproj_unit(opw, u):
            m = opw * 4 + u // 2
            if u % 2 == 0:
                stages[m] = opool.tile([128, E], F32, tag="ost", name=f"st{m}")
            outproj_half(m, u % 2, stages[m])

        for opw in range(3):
            for u in range(4):
                extras_post[(2 * opw + 2) * 16 + 12 + u].append(
                    lambda opw=opw, u=u: outproj_unit(opw, u))
            for u in range(4, 8):
                extras_post[(2 * opw + 3) * 16 + 2 * (u - 4) + 1].append(
                    lambda opw=opw, u=u: outproj_unit(opw, u))

        # OV emission: per-pair lag shrinks toward the end so the last
        # pair's PV work lands almost entirely in-stream; transitions
        # auto-bunch to 2 groups/slot.  finalize(pp) sits between pairs.
        LAGS = [10, 10, 10, 10, 8, 6, 4, 2]
        ovmap = defaultdict(list)     # g -> ordered ops: ("ov", n) | ("fin", pp)
        tail_ops = []
        gcur = 0
        for pp in range(8):
            for kc in range(16):
                n = pp * 16 + kc
                g = max(pp * 16 + kc + LAGS[pp], gcur)
                while g <= 127 and sum(
                        1 for o in ovmap[g] if o[0] == "ov") >= 2:
                    g += 1
                if g > 127:
                    tail_ops.append(("ov", n))
                else:
                    ovmap[g].append(("ov", n))
                    gcur = g
            if pp < 7:
                fg = gcur + 1 if not tail_ops else None
                if fg is not None and fg <= 127:
                    ovmap[fg].insert(0, ("fin", pp))
                    gcur = fg   # next pair's groups must follow the fin
                else:
                    tail_ops.append(("fin", pp))
        n_total = 128 - len([o for o in tail_ops if o[0] == "ov"])

        # ---- warmup: anchor the PE p-state ramp while DMAs stream ------
        wu_a = consts.tile([128, 128], FP16)
        wu_b = consts.tile([128, 64], FP16)
        nc.vector.memset(wu_a[:], 0.0)
        nc.vector.memset(wu_b[:], 0.0)
        for i in range(3):
            ps = psX.tile([128, 512], F32, tag="px", name=f"wu{i}")
            nc.tensor.matmul(ps[:, 0:64], wu_a[:], wu_b[:], start=True, stop=True)

        wproj1(wk_sb, xk_blks[0], kT_sb, 0, 0, use_act=True)
        wproj1(wk_sb, xk_blks[0], kT_sb, 0, 1, use_act=True)
        wproj1(wq_sb, xq_blks[0], qT_sb, 0, 0, use_act=True)
        wproj1(wq_sb, xq_blks[0], qT_sb, 0, 1, use_act=True)

        pts = {}
        ovts = None
        for g in range(128):
            p, kc = divmod(g, 16)
            w, pair = divmod(p, 2)
            qs = slice(w * 512, (w + 1) * 512)
            ks = slice(kc * 128, (kc + 1) * 128)

            ps_s = psS.tile([128, 1024], F32)
            nc.tensor.matmul(ps_s[:, 0:512],
                             kT_sb[0:64, pair, ks],
                             qT_sb[0:64, pair, qs],
                             start=True, stop=True)
            nc.tensor.matmul(ps_s[:, 512:1024],
                             kT_sb[64:128, pair, ks],
                             qT_sb[64:128, pair, qs],
                             start=True, stop=True)
            pT = ptpool.tile([128, 1024], FP16, tag="pT")
            nc.scalar.activation(pT[:], ps_s[:],
                                 mybir.ActivationFunctionType.Exp,
                                 scale=0.125)
            pts[g] = pT

            for thunk in extras_pre.get(g, ()):
                thunk()
            for op, val in ovmap.get(g, ()):
                if op == "fin":
                    finalize(ovts, val // 2, val % 2)
                    continue
                pp, pkc = divmod(val, 16)
                if pkc == 0:
                    ovA = psOV.tile([128, 2, 2, 65], F32, tag="ovA",
                                    name=f"ovA{pp}")
                    ovB = psOV.tile([128, 2, 2, 65], F32, tag="ovB",
                                    name=f"ovB{pp}")
                    ovts = ((ovA, (0, 1)), (ovB, (2, 3)))
                ov_group(ovts, pp % 2, pts.pop(val), pkc)
            for thunk in extras_post.get(g, ()):
                thunk()

        # ---- tail: leftover OV/fin ops, then phase-ordered finalize +
        # outproj; normalize split across DVE/ACT, scores PSUM banks reused
        # for extra outproj accumulators so the 8 halves pipeline deeply.
        for op, val in tail_ops:
            if op == "fin":
                finalize(ovts, val // 2, val % 2)
            else:
                pp, pkc = divmod(val, 16)
                if pkc == 0:
                    ovA = psOV.tile([128, 2, 2, 65], F32, tag="ovA",
                                    name=f"ovA{pp}")
                    ovB = psOV.tile([128, 2, 2, 65], F32, tag="ovB",
                                    name=f"ovB{pp}")
                    ovts = ((ovA, (0, 1)), (ovB, (2, 3)))
                ov_group(ovts, pp % 2, pts.pop(val), pkc)
        psA = psS.tile([128, 1024], F32, tag="ps_s", name="psA")
        psB = psS.tile([128, 1024], F32, tag="ps_s", name="psB")
        for ql in range(4):
            ovt = ovts[0] if ql < 2 else ovts[1]
            o_n = onpool.tile([128, 128], FP16, tag="on")
            for h2 in range(2):
                rt = rpool.tile([128, 1], F32, tag="rt")
                nc.vector.reciprocal(rt[:], ovt[0][:, ql % 2, h2, 64:65])
                if h2 == 0:
                    nc.vector.tensor_scalar_mul(
                        o_n[:, 0:64], ovt[0][:, ql % 2, 0, 0:64], rt[:])
                else:
                    nc.scalar.activation(
                        o_n[:, 64:128], ovt[0][:, ql % 2, 1, 0:64],
                        mybir.ActivationFunctionType.Copy, scale=rt[:])
            # PE transpose (53ns) beats the crossbar DMA round-trip here;
            # scratch lives in the dead scores tiles' second zero-region
            tsrc = (psA, psB)[ql // 2]
            off = 768 + (ql % 2) * 64
            tps = tsrc[:, off:off + 64].bitcast(FP16)
            nc.tensor.transpose(tps, o_n[:], id_sb[:])
            qs = slice(3 * 512 + ql * 128, 3 * 512 + (ql + 1) * 128)
            nc.vector.tensor_copy(oT_sb[:, 1, qs], tps)
        for ql in range(4):
            m = 12 + ql
            for j in range(2):
                if j == 0:
                    ps = (psA, psB)[ql % 2][:, 0:512]
                else:
                    pst = psX.tile([128, 512], F32, tag="px",
                                   name=f"tp{m}")
                    ps = pst[:]
                for jp in range(2):
                    nc.tensor.matmul(
                        ps,
                        oT_sb[:, jp, m * 128:(m + 1) * 128],
                        wo_sb[:, jp, j * 512:(j + 1) * 512],
                        start=(jp == 0), stop=(jp == 1))
                half = opool.tile([128, 512], FP16, tag="osth",
                                  name=f"sth{m}_{j}")
                if j == 0:
                    nc.vector.tensor_copy(half[:], ps)
                else:
                    nc.scalar.copy(half[:], ps)
                dmaeng = nc.sync if j == 0 else nc.scalar
                dmaeng.dma_start(
                    out16[(m - 12) * 128:(m - 11) * 128,
                          j * 512:(j + 1) * 512],
                    half[:])

    return nc


_NC_CACHE = {}


def _get_nc():
    if "nc" not in _NC_CACHE:
        _NC_CACHE["nc"] = build()
    return _NC_CACHE["nc"]


def _shard_inputs(query, key, value, Wq, Wk, Wv, Wo):
    """Host-side sharding + layout prep: core c = (batch c//4, head-group c%4)."""
    f16 = np.float16
    xT = []
    for b in range(B):
        xT.append((
            np.ascontiguousarray(query[b].T).astype(f16),
            np.ascontiguousarray(key[b].T).astype(f16),
            np.ascontiguousarray(value[b].T).astype(f16),
        ))
    ident = np.eye(128, dtype=f16)
    wT = []
    for g in range(4):
        gc = slice(g * 256, (g + 1) * 256)
        wT.append((
            np.ascontiguousarray(Wq[gc].T).astype(f16),
            np.ascontiguousarray(Wk[gc].T).astype(f16),
            np.ascontiguousarray(Wv[gc].T).astype(f16),
            np.ascontiguousarray(Wo[:, gc].T).astype(f16),
        ))
    in_maps = []
    for c in range(NCORES):
        b, g = c // 4, c % 4
        qT, kT, vT = xT[b]
        wq, wk, wv, wo = wT[g]
        in_maps.append({
            "xqT": qT, "xkT": kT, "xvT": vT,
            "wqT": wq, "wkT": wk, "wvT": wv, "woT": wo,
            "identT": ident,
        })
    return in_maps


def kernel(query, key, value, Wq, Wk, Wv, Wo):
    query = np.asarray(query, dtype=np.float32)
    key = np.asarray(key, dtype=np.float32)
    value = np.asarray(value, dtype=np.float32)
    Wq = np.asarray(Wq, dtype=np.float32)
    Wk = np.asarray(Wk, dtype=np.float32)
    Wv = np.asarray(Wv, dtype=np.float32)
    Wo = np.asarray(Wo, dtype=np.float32)

    nc = _get_nc()
    in_maps = _shard_inputs(query, key, value, Wq, Wk, Wv, Wo)
    res = run_bass_kernel_spmd(nc, in_maps, core_ids=list(range(NCORES)))

    out = np.zeros((B, S, E), dtype=np.float32)
    for c in range(NCORES):
        out[c // 4][0:1536] += res.results[c]["out"][0:1536]
        out[c // 4][1536:2048] += res.results[c]["out16"].astype(np.float32)
    return out


# revision 19
# speedup vs baseline: 1.0035x; 1.0035x over previous
"""Trainium2 Bass kernel for nn_MultiHeadAttention (B=2, S=2048, E=1024, H=16).

Sharding: 8 NeuronCores = data-parallel over the 2 batches x tensor-parallel
over the 16 heads in 4 groups of 4 heads (Wq/Wk/Wv split column-wise, Wo
row-wise).  Each core computes a full-[S, E] partial of its batch's output;
the host sums the 4 head-group partials per batch.

Per-core pipeline (ACT exp stream is the pace-setter at ~133us):
  Q.T/K.T[n, s]  per 512-col window: psX-accumulated e-outer matmuls
                 chasing column-block DMAs (first exp at ~11us).
  S.T_h[k, q]    [64,128]x[64,512] per (window, head-pair, k-chunk),
                 head pair packed in one [128,1024] PSUM tile.
  P.T            one ACT exp op per (window, pair, k-chunk); fp16 out.
  O[q, d|sum]    TRANSPOSED P.V: stationary = P.T chunk [128k,128q],
                 moving = [V_h | ones] [128k, 65] -> accumulates
                 [128q, 65] in sub-bank PSUM slices (2x fewer PE rows
                 than the O.T orientation).  Lagged one (window,pair)
                 behind the exp stream.
  normalize      DVE reciprocal of col 64 + per-partition scalar mult.
  O.T            crossbar DMA transpose [128q,128hd] -> [128hd,128q].
  out[m, :]      stationary oT chunk [128 hd, 128 m] x moving Wo.T
                 [128 hd, 512 e], 2-chunk contraction, DVE copy + SWDGE
                 store.

dtypes: all matmul operands fp16 (1 cyc/row on PE); PSUM accumulation fp32;
softmax exp on fp32 scores.
"""

import numpy as np
from contextlib import ExitStack

import concourse.bass as bass
import concourse.mybir as mybir
import concourse.tile as tile
from concourse.tile import ScopedClock
from concourse.bass_utils import run_bass_kernel_spmd

# ---------------------------------------------------------------------------
# Workarounds for the walrus build on this stack, which rejects more than ONE
# semaphore wait per instruction ("Too many sync wait commands").
# ---------------------------------------------------------------------------
_orig_commit_instruction = tile.TileContext._commit_instruction


def _commit_instruction(self, inst, lazy_reg_writes=True):
    si = getattr(inst, "sync_info", None)
    if si is not None and si.on_wait and len(si.on_wait) > 1:
        waits = list(si.on_wait)
        for w in waits[:-1]:
            nop = mybir.InstNoOp(
                name=self.nc.get_next_instruction_name(),
                ins=[], outs=[], engine=inst.engine,
            )
            nop.bass_nofuse = True
            nop.sync_info = mybir.SyncInfo(on_wait=[w], on_update=[])
            _orig_commit_instruction(self, nop, lazy_reg_writes=False)
        inst.sync_info = mybir.SyncInfo(
            on_wait=[waits[-1]], on_update=list(si.on_update or [])
        )
    return _orig_commit_instruction(self, inst, lazy_reg_writes)


def _drain_and_barrier(self, tick_clock, wait_clock):
    nc = self.nc
    drain_inst = nc.sync.drain()
    wait_clock.add_sem_waits(
        drain_inst.ins, ScopedClock({None: tick_clock.global_clock})
    )
    si = drain_inst.ins.sync_info
    waits = list(si.on_wait) if si and si.on_wait else []
    if len(waits) > 1:
        drain_inst.ins.sync_info = mybir.SyncInfo(
            on_wait=waits[:1], on_update=list(si.on_update or [])
        )
        for w in waits[1:]:
            extra = nc.sync.drain()
            esi = extra.ins.sync_info
            extra.ins.sync_info = mybir.SyncInfo(
                on_wait=[w],
                on_update=list(esi.on_update or []) if esi else [],
            )
    nc.all_engine_barrier()
    assert self.sems is not None
    popped = nc._tile_sem_poison_stack.pop()
    assert popped is self._sem_poison
    nc.clear_and_free_semaphores(list(self.sems.allocated().values()))
    nc.all_engine_barrier()


def _apply_tilefix():
    tile.TileContext._commit_instruction = _commit_instruction
    tile.TileContext._drain_and_barrier = _drain_and_barrier


_apply_tilefix()

# ---------------------------------------------------------------------------
# Problem constants (hardcoded)
# ---------------------------------------------------------------------------
B, S, E, H = 2, 2048, 1024, 16
HC, D = 4, 64              # heads per core, head dim
NCORES = 8
NE = E // 128              # 8  e-chunks
NW = S // 512              # 4  q/k windows
NK = S // 128              # 16 k-chunks
NM = S // 128              # 16 m-chunks

F32 = mybir.dt.float32
FP16 = mybir.dt.float16


def build(ptbufs=18):
    nc = bass.Bass()
    xqT = nc.dram_tensor("xqT", [E, S], FP16, kind="ExternalInput")
    xkT = nc.dram_tensor("xkT", [E, S], FP16, kind="ExternalInput")
    xvT = nc.dram_tensor("xvT", [E, S], FP16, kind="ExternalInput")
    wqT = nc.dram_tensor("wqT", [E, 256], FP16, kind="ExternalInput")
    wkT = nc.dram_tensor("wkT", [E, 256], FP16, kind="ExternalInput")
    wvT = nc.dram_tensor("wvT", [E, 256], FP16, kind="ExternalInput")
    woT = nc.dram_tensor("woT", [256, E], FP16, kind="ExternalInput")
    out = nc.dram_tensor("out", [S, E], F32, kind="ExternalOutput")
    out16 = nc.dram_tensor("out16", [512, E], FP16, kind="ExternalOutput")
    identT = nc.dram_tensor("identT", [128, 128], FP16, kind="ExternalInput")

    with tile.TileContext(nc) as tc, ExitStack() as ctx:
        consts = ctx.enter_context(tc.tile_pool(name="consts", bufs=1))
        wpool = ctx.enter_context(tc.tile_pool(name="w", bufs=1))
        actpool = ctx.enter_context(tc.tile_pool(name="acts", bufs=1))
        xkpool = ctx.enter_context(tc.tile_pool(name="xk", bufs=3))
        xqpool = ctx.enter_context(tc.tile_pool(name="xq", bufs=3))
        xvpool = ctx.enter_context(tc.tile_pool(name="xv", bufs=3))
        ptpool = ctx.enter_context(tc.tile_pool(name="pT", bufs=ptbufs))
        onpool = ctx.enter_context(tc.tile_pool(name="on", bufs=4))
        rpool = ctx.enter_context(tc.tile_pool(name="recip", bufs=8))
        opool = ctx.enter_context(tc.tile_pool(name="outstage", bufs=2))
        psS = ctx.enter_context(tc.tile_pool(name="psS", bufs=2, space="PSUM"))
        psOV = ctx.enter_context(tc.tile_pool(name="psOV", bufs=1, space="PSUM"))
        psX = ctx.enter_context(tc.tile_pool(name="psX", bufs=2, space="PSUM"))

        # preload the exp table before the hot loop
        dummy = consts.tile([1, 8], F32)
        nc.vector.memset(dummy[:], 0.0)
        nc.scalar.activation(dummy[:], dummy[:], mybir.ActivationFunctionType.Exp)

        wq_sb = wpool.tile([128, NE, 256], FP16)
        wk_sb = wpool.tile([128, NE, 256], FP16)
        wv_sb = wpool.tile([128, NE, 256], FP16)
        wo_sb = wpool.tile([128, 2, E], FP16)

        qT_sb = actpool.tile([128, 2, S], FP16)        # [(2 heads x d), pair, s]
        kT_sb = actpool.tile([128, 2, S], FP16)
        v_sb = actpool.tile([128, NK, HC, 65], FP16)   # [s%128, k, h, V_h|ones]
        oT_sb = actpool.tile([128, 2, S], FP16, name="oT")  # [(h2 d), pair, s]

        nc.vector.memset(v_sb[:, :, :, 64:65], 1.0)

        # ---- DMA emission order on the SP queue (arrival order == need) ---
        def colblock(x, j):
            return x[:, j * 512:(j + 1) * 512].rearrange(
                "(ec p) s -> p ec s", p=128)

        def halfblock(x, j, h):
            return x[h * 512:(h + 1) * 512,
                     j * 512:(j + 1) * 512].rearrange(
                "(ec p) s -> p ec s", p=128)

        xq_blks = {}
        xk_blks = {}
        xv_blks = {}

        def load_x(pool, src, blks, j, tag, split=False):
            t = pool.tile([128, NE, 512], FP16, tag=tag, name=f"{tag}{j}")
            if split:
                # two half-e DMAs so the projection can chase the first half
                nc.sync.dma_start(t[:, 0:4, :], halfblock(src, j, 0))
                nc.sync.dma_start(t[:, 4:8, :], halfblock(src, j, 1))
            else:
                nc.sync.dma_start(t[:], colblock(src, j))
            blks[j] = t

        # arrival order == need order (single serialized DMA device)
        nc.sync.dma_start(wk_sb[:], wkT.rearrange("(ec p) n -> p ec n", p=128))
        load_x(xkpool, xkT, xk_blks, 0, "xkb", split=True)
        nc.sync.dma_start(wq_sb[:], wqT.rearrange("(ec p) n -> p ec n", p=128))
        load_x(xqpool, xqT, xq_blks, 0, "xqb", split=True)
        for j in range(1, NW):
            load_x(xkpool, xkT, xk_blks, j, "xkb", split=True)
        nc.sync.dma_start(wv_sb[:], wvT.rearrange("(ec p) n -> p ec n", p=128))
        load_x(xvpool, xvT, xv_blks, 0, "xvb")
        load_x(xvpool, xvT, xv_blks, 1, "xvb")
        load_x(xqpool, xqT, xq_blks, 1, "xqb")
        load_x(xvpool, xvT, xv_blks, 2, "xvb")
        load_x(xvpool, xvT, xv_blks, 3, "xvb")
        load_x(xqpool, xqT, xq_blks, 2, "xqb")
        load_x(xqpool, xqT, xq_blks, 3, "xqb")
        nc.sync.dma_start(wo_sb[:], woT.rearrange("(j p) e -> p j e", p=128))
        id_sb = wpool.tile([128, 128], FP16)
        nc.sync.dma_start(id_sb[:], identT[:, :])

        # ---- building blocks -------------------------------------------
        def wproj(w_sb, blk, dst, win):
            """project one 512-col window of x into dst[:, :, win] (2 psX)."""
            ws = slice(win * 512, (win + 1) * 512)
            for nch in range(2):
                ps = psX.tile([128, 512], F32, tag="px", name=f"pj{win}_{nch}")
                for e in range(NE):
                    nc.tensor.matmul(
                        ps[:],
                        w_sb[:, e, nch * 128:(nch + 1) * 128],
                        blk[:, e, :],
                        start=(e == 0), stop=(e == NE - 1))
                nc.vector.tensor_copy(dst[:, nch, ws], ps[:])

        def vproj(m):
            blk = xv_blks[m // 4]
            ps = psX.tile([128, 512], F32, tag="px", name=f"vp{m}")
            for e in range(NE):
                nc.tensor.matmul(
                    ps[:, 0:256],
                    blk[:, e, (m % 4) * 128:(m % 4 + 1) * 128],
                    wv_sb[:, e, :],
                    start=(e == 0), stop=(e == NE - 1))
            nc.vector.tensor_copy(
                v_sb[:, m, :, 0:64],
                ps[:, 0:256].rearrange("p (h c) -> p h c", h=HC))

        def ov_group(ovts, pair, pT, kc):
            """8 transposed-PV matmuls for one k-chunk; sub-bank psum accum."""
            for t_i, (ovt, qlocs) in enumerate(ovts):
                for si, (h2, ql) in enumerate(
                        [(h, q) for h in range(2) for q in qlocs]):
                    nc.tensor.matmul(
                        ovt[:, ql % 2, h2, :],
                        pT[:, h2 * 512 + ql * 128: h2 * 512 + (ql + 1) * 128],
                        v_sb[:, kc, 2 * pair + h2, :],
                        start=(kc == 0 and si == 0),
                        stop=(kc == NK - 1 and si == 3),
                        skip_group_check=True)

        def finalize(ovts, w, pair):
            """normalize + crossbar-transpose one (window, pair)."""
            for ovt, qlocs in ovts:
                for ql in qlocs:
                    o_n = onpool.tile([128, 128], FP16, tag="on")
                    for h2 in range(2):
                        rt = rpool.tile([128, 1], F32, tag="rt")
                        nc.vector.reciprocal(rt[:], ovt[:, ql % 2, h2, 64:65])
                        nc.vector.tensor_scalar_mul(
                            o_n[:, h2 * 64:(h2 + 1) * 64],
                            ovt[:, ql % 2, h2, 0:64],
                            rt[:])
                    qs = slice(w * 512 + ql * 128, w * 512 + (ql + 1) * 128)
                    nc.sync.dma_start_transpose(oT_sb[:, pair, qs], o_n[:])

        def outproj_half(m, j, stage):
            ps = psX.tile([128, 512], F32, tag="px", name=f"op{m}_{j}")
            for jp in range(2):
                nc.tensor.matmul(
                    ps[:],
                    oT_sb[:, jp, m * 128:(m + 1) * 128],
                    wo_sb[:, jp, j * 512:(j + 1) * 512],
                    start=(jp == 0), stop=(jp == 1))
            nc.vector.tensor_copy(stage[:, j * 512:(j + 1) * 512], ps[:])
            if j == 1:
                nc.gpsimd.dma_start(out[m * 128:(m + 1) * 128, :], stage[:])

        # ---- global-slot schedule --------------------------------------
        # slot g = p*16 + kc carries scores(p,kc)+exp; OV work is lagged
        # OVLAG slots behind the exp stream (rolling across p boundaries).
        OVLAG = 10
        TAILOV = 8            # OV groups left for the post-stream tail
        from collections import defaultdict
        extras_pre = defaultdict(list)    # g -> thunks (feeders: proj work)
        extras_post = defaultdict(list)   # g -> thunks (drains: outproj)

        def sched_wproj(g, w_sb, blks, dst, win, nch=None):
            for n in ((0, 1) if nch is None else (nch,)):
                extras_pre[g].append(
                    lambda n=n, win=win: wproj1(w_sb, blks[win], dst, win, n))

        def wproj1(w_sb, blk, dst, win, nch, use_act=False):
            ws = slice(win * 512, (win + 1) * 512)
            ps = psX.tile([128, 512], F32, tag="px", name=f"pj{win}_{nch}")
            for e in range(NE):
                nc.tensor.matmul(
                    ps[:],
                    w_sb[:, e, nch * 128:(nch + 1) * 128],
                    blk[:, e, :],
                    start=(e == 0), stop=(e == NE - 1))
            if use_act:
                nc.scalar.copy(dst[:, nch, ws], ps[:])
            else:
                nc.vector.tensor_copy(dst[:, nch, ws], ps[:])

        # K windows 1-3 early in p0 (chasing the xk block DMAs)
        sched_wproj(2, wk_sb, xk_blks, kT_sb, 1, nch=0)
        sched_wproj(3, wk_sb, xk_blks, kT_sb, 1, nch=1)
        sched_wproj(6, wk_sb, xk_blks, kT_sb, 2, nch=0)
        sched_wproj(7, wk_sb, xk_blks, kT_sb, 2, nch=1)
        sched_wproj(10, wk_sb, xk_blks, kT_sb, 3, nch=0)
        sched_wproj(11, wk_sb, xk_blks, kT_sb, 3, nch=1)
        # V tiles 2 slots ahead of their OV consumer (max arrival slack)
        for m in range(NM):
            extras_pre[m + 10].append(lambda m=m: vproj(m))
        # remaining Q windows, off the critical path, in slack windows
        sched_wproj(18, wq_sb, xq_blks, qT_sb, 1, nch=0)
        sched_wproj(20, wq_sb, xq_blks, qT_sb, 1, nch=1)
        sched_wproj(40, wq_sb, xq_blks, qT_sb, 2, nch=0)
        sched_wproj(56, wq_sb, xq_blks, qT_sb, 2, nch=1)
        sched_wproj(72, wq_sb, xq_blks, qT_sb, 3, nch=0)
        sched_wproj(88, wq_sb, xq_blks, qT_sb, 3, nch=1)

        # outproj window w' after both its finalizes: units u0..3 at
        # p(2w'+2) slots 12..15, u4..7 at p(2w'+3) slots 1,3,5,7
        stages = {}

        def outproj_unit(opw, u):
            m = opw * 4 + u // 2
            if u % 2 == 0:
                stages[m] = opool.tile([128, E], F32, tag="ost", name=f"st{m}")
            outproj_half(m, u % 2, stages[m])

        for opw in range(3):
            for u in range(4):
                extras_post[(2 * opw + 2) * 16 + 12 + u].append(
                    lambda opw=opw, u=u: outproj_unit(opw, u))
            for u in range(4, 8):
                extras_post[(2 * opw + 3) * 16 + 2 * (u - 4) + 1].append(
                    lambda opw=opw, u=u: outproj_unit(opw, u))

        # OV emission: per-pair lag shrinks toward the end so the last
        # pair's PV work lands almost entirely in-stream; transitions
        # auto-bunch to 2 groups/slot.  finalize(pp) sits between pairs.
        LAGS = [10, 10, 10, 10, 8, 6, 4, 2]
        ovmap = defaultdict(list)     # g -> ordered ops: ("ov", n) | ("fin", pp)
        tail_ops = []
        gcur = 0
        for pp in range(8):
            for kc in range(16):
                n = pp * 16 + kc
                g = max(pp * 16 + kc + LAGS[pp], gcur)
                while g <= 127 and sum(
                        1 for o in ovmap[g] if o[0] == "ov") >= 2:
                    g += 1
                if g > 127:
                    tail_ops.append(("ov", n))
                else:
                    ovmap[g].append(("ov", n))
                    gcur = g
            if pp < 7:
                fg = gcur + 1 if not tail_ops else None
                if fg is not None and fg <= 127:
                    ovmap[fg].insert(0, ("fin", pp))
                    gcur = fg   # next pair's groups must follow the fin
                else:
                    tail_ops.append(("fin", pp))
        n_total = 128 - len([o for o in tail_ops if o[0] == "ov"])

        # ---- warmup: anchor the PE p-state ramp while DMAs stream ------
        wu_a = consts.tile([128, 128], FP16)
        wu_b = consts.tile([128, 64], FP16)
        nc.vector.memset(wu_a[:], 0.0)
        nc.vector.memset(wu_b[:], 0.0)
        for i in range(3):
            ps = psX.tile([128, 512], F32, tag="px", name=f"wu{i}")
            nc.tensor.matmul(ps[:, 0:64], wu_a[:], wu_b[:], start=True, stop=True)

        wproj1(wk_sb, xk_blks[0], kT_sb, 0, 0, use_act=True)
        wproj1(wk_sb, xk_blks[0], kT_sb, 0, 1, use_act=True)
        wproj1(wq_sb, xq_blks[0], qT_sb, 0, 0, use_act=True)
        wproj1(wq_sb, xq_blks[0], qT_sb, 0, 1, use_act=True)

        pts = {}
        ovts = None
        for g in range(128):
            p, kc = divmod(g, 16)
            w, pair = divmod(p, 2)
            qs = slice(w * 512, (w + 1) * 512)
            ks = slice(kc * 128, (kc + 1) * 128)

            ps_s = psS.tile([128, 1024], F32)
            nc.tensor.matmul(ps_s[:, 0:512],
                             kT_sb[0:64, pair, ks],
                             qT_sb[0:64, pair, qs],
                             start=True, stop=True)
            nc.tensor.matmul(ps_s[:, 512:1024],
                             kT_sb[64:128, pair, ks],
                             qT_sb[64:128, pair, qs],
                             start=True, stop=True)
            pT = ptpool.tile([128, 1024], FP16, tag="pT")
            nc.scalar.activation(pT[:], ps_s[:],
                                 mybir.ActivationFunctionType.Exp,
                                 scale=0.125)
            pts[g] = pT

            for thunk in extras_pre.get(g, ()):
                thunk()
            for op, val in ovmap.get(g, ()):
                if op == "fin":
                    finalize(ovts, val // 2, val % 2)
                    continue
                pp, pkc = divmod(val, 16)
                if pkc == 0:
                    ovA = psOV.tile([128, 2, 2, 65], F32, tag="ovA",
                                    name=f"ovA{pp}")
                    ovB = psOV.tile([128, 2, 2, 65], F32, tag="ovB",
                                    name=f"ovB{pp}")
                    ovts = ((ovA, (0, 1)), (ovB, (2, 3)))
                ov_group(ovts, pp % 2, pts.pop(val), pkc)
            for thunk in extras_post.get(g, ()):
                thunk()

        # ---- tail: leftover OV/fin ops, then phase-ordered finalize +
        # outproj; normalize split across DVE/ACT, scores PSUM banks reused
        # for extra outproj accumulators so the 8 halves pipeline deeply.
        for op, val in tail_ops:
            if op == "fin":
                finalize(ovts, val // 2, val % 2)
            else:
                pp, pkc = divmod(val, 16)
                if pkc == 0:
                    ovA = psOV.tile([128, 2, 2, 65], F32, tag="ovA",
                                    name=f"ovA{pp}")
                    ovB = psOV.tile([128, 2, 2, 65], F32, tag="ovB",
                                    name=f"ovB{pp}")
                    ovts = ((ovA, (0, 1)), (ovB, (2, 3)))
                ov_group(ovts, pp % 2, pts.pop(val), pkc)
        psA = psS.tile([128, 1024], F32, tag="ps_s", name="psA")
        psB = psS.tile([128, 1024], F32, tag="ps_s", name="psB")
        for ql in range(4):
            ovt = ovts[0] if ql < 2 else ovts[1]
            o_n = onpool.tile([128, 128], FP16, tag="on")
            for h2 in range(2):
                rt = rpool.tile([128, 1], F32, tag="rt")
                nc.vector.reciprocal(rt[:], ovt[0][:, ql % 2, h2, 64:65])
                if h2 == 0:
                    nc.vector.tensor_scalar_mul(
                        o_n[:, 0:64], ovt[0][:, ql % 2, 0, 0:64], rt[:])
                else:
                    nc.scalar.activation(
                        o_n[:, 64:128], ovt[0][:, ql % 2, 1, 0:64],
                        mybir.ActivationFunctionType.Copy, scale=rt[:])
            # PE transpose (53ns) beats the crossbar DMA round-trip here;
            # scratch lives in the dead scores tiles' second zero-region
            tsrc = (psA, psB)[ql // 2]
            off = 768 + (ql % 2) * 64
            tps = tsrc[:, off:off + 64].bitcast(FP16)
            nc.tensor.transpose(tps, o_n[:], id_sb[:])
            qs = slice(3 * 512 + ql * 128, 3 * 512 + (ql + 1) * 128)
            nc.vector.tensor_copy(oT_sb[:, 1, qs], tps)
        for ql in range(4):
            m = 12 + ql
            for j in range(2):
                if j == 0:
                    ps = (psA, psB)[ql % 2][:, 0:512]
                else:
                    pst = psX.tile([128, 512], F32, tag="px",
                                   name=f"tp{m}")
                    ps = pst[:]
                for jp in range(2):
                    nc.tensor.matmul(
                        ps,
                        oT_sb[:, jp, m * 128:(m + 1) * 128],
                        wo_sb[:, jp, j * 512:(j + 1) * 512],
                        start=(jp == 0), stop=(jp == 1))
                half = opool.tile([128, 512], FP16, tag="osth",
                                  name=f"sth{m}_{j}")
                if j == 0:
                    nc.vector.tensor_copy(half[:], ps)
                else:
                    nc.scalar.copy(half[:], ps)
                nc.sync.dma_start(
                    out16[(m - 12) * 128:(m - 11) * 128,
                          j * 512:(j + 1) * 512],
                    half[:])

    return nc


_NC_CACHE = {}


def _get_nc():
    if "nc" not in _NC_CACHE:
        _NC_CACHE["nc"] = build()
    return _NC_CACHE["nc"]


def _shard_inputs(query, key, value, Wq, Wk, Wv, Wo):
    """Host-side sharding + layout prep: core c = (batch c//4, head-group c%4)."""
    f16 = np.float16
    xT = []
    for b in range(B):
        xT.append((
            np.ascontiguousarray(query[b].T).astype(f16),
            np.ascontiguousarray(key[b].T).astype(f16),
            np.ascontiguousarray(value[b].T).astype(f16),
        ))
    ident = np.eye(128, dtype=f16)
    wT = []
    for g in range(4):
        gc = slice(g * 256, (g + 1) * 256)
        wT.append((
            np.ascontiguousarray(Wq[gc].T).astype(f16),
            np.ascontiguousarray(Wk[gc].T).astype(f16),
            np.ascontiguousarray(Wv[gc].T).astype(f16),
            np.ascontiguousarray(Wo[:, gc].T).astype(f16),
        ))
    in_maps = []
    for c in range(NCORES):
        b, g = c // 4, c % 4
        qT, kT, vT = xT[b]
        wq, wk, wv, wo = wT[g]
        in_maps.append({
            "xqT": qT, "xkT": kT, "xvT": vT,
            "wqT": wq, "wkT": wk, "wvT": wv, "woT": wo,
            "identT": ident,
        })
    return in_maps


def kernel(query, key, value, Wq, Wk, Wv, Wo):
    query = np.asarray(query, dtype=np.float32)
    key = np.asarray(key, dtype=np.float32)
    value = np.asarray(value, dtype=np.float32)
    Wq = np.asarray(Wq, dtype=np.float32)
    Wk = np.asarray(Wk, dtype=np.float32)
    Wv = np.asarray(Wv, dtype=np.float32)
    Wo = np.asarray(Wo, dtype=np.float32)

    nc = _get_nc()
    in_maps = _shard_inputs(query, key, value, Wq, Wk, Wv, Wo)
    res = run_bass_kernel_spmd(nc, in_maps, core_ids=list(range(NCORES)))

    out = np.zeros((B, S, E), dtype=np.float32)
    for c in range(NCORES):
        out[c // 4][0:1536] += res.results[c]["out"][0:1536]
        out[c // 4][1536:2048] += res.results[c]["out16"].astype(np.float32)
    return out


# revision 20
# speedup vs baseline: 1.0323x; 1.0287x over previous
"""Trainium2 Bass kernel for nn_MultiHeadAttention (B=2, S=2048, E=1024, H=16).

Sharding: 8 NeuronCores = data-parallel over the 2 batches x tensor-parallel
over the 16 heads in 4 groups of 4 heads (Wq/Wk/Wv split column-wise, Wo
row-wise).  Each core computes a full-[S, E] partial of its batch's output;
the host sums the 4 head-group partials per batch.

Per-core pipeline (ACT exp stream is the pace-setter at ~133us):
  Q.T/K.T[n, s]  per 512-col window: psX-accumulated e-outer matmuls
                 chasing column-block DMAs (first exp at ~11us).
  S.T_h[k, q]    [64,128]x[64,512] per (window, head-pair, k-chunk),
                 head pair packed in one [128,1024] PSUM tile.
  P.T            one ACT exp op per (window, pair, k-chunk); fp16 out.
  O[q, d|sum]    TRANSPOSED P.V: stationary = P.T chunk [128k,128q],
                 moving = [V_h | ones] [128k, 65] -> accumulates
                 [128q, 65] in sub-bank PSUM slices (2x fewer PE rows
                 than the O.T orientation).  Lagged one (window,pair)
                 behind the exp stream.
  normalize      DVE reciprocal of col 64 + per-partition scalar mult.
  O.T            crossbar DMA transpose [128q,128hd] -> [128hd,128q].
  out[m, :]      stationary oT chunk [128 hd, 128 m] x moving Wo.T
                 [128 hd, 512 e], 2-chunk contraction, DVE copy + SWDGE
                 store.

dtypes: all matmul operands fp16 (1 cyc/row on PE); PSUM accumulation fp32;
softmax exp on fp32 scores.
"""

import numpy as np
from contextlib import ExitStack

import concourse.bass as bass
import concourse.mybir as mybir
import concourse.tile as tile
from concourse.tile import ScopedClock
from concourse.bass_utils import run_bass_kernel_spmd

# ---------------------------------------------------------------------------
# Workarounds for the walrus build on this stack, which rejects more than ONE
# semaphore wait per instruction ("Too many sync wait commands").
# ---------------------------------------------------------------------------
_orig_commit_instruction = tile.TileContext._commit_instruction


def _commit_instruction(self, inst, lazy_reg_writes=True):
    si = getattr(inst, "sync_info", None)
    if si is not None and si.on_wait and len(si.on_wait) > 1:
        waits = list(si.on_wait)
        for w in waits[:-1]:
            nop = mybir.InstNoOp(
                name=self.nc.get_next_instruction_name(),
                ins=[], outs=[], engine=inst.engine,
            )
            nop.bass_nofuse = True
            nop.sync_info = mybir.SyncInfo(on_wait=[w], on_update=[])
            _orig_commit_instruction(self, nop, lazy_reg_writes=False)
        inst.sync_info = mybir.SyncInfo(
            on_wait=[waits[-1]], on_update=list(si.on_update or [])
        )
    return _orig_commit_instruction(self, inst, lazy_reg_writes)


def _drain_and_barrier(self, tick_clock, wait_clock):
    nc = self.nc
    drain_inst = nc.sync.drain()
    wait_clock.add_sem_waits(
        drain_inst.ins, ScopedClock({None: tick_clock.global_clock})
    )
    si = drain_inst.ins.sync_info
    waits = list(si.on_wait) if si and si.on_wait else []
    if len(waits) > 1:
        drain_inst.ins.sync_info = mybir.SyncInfo(
            on_wait=waits[:1], on_update=list(si.on_update or [])
        )
        for w in waits[1:]:
            extra = nc.sync.drain()
            esi = extra.ins.sync_info
            extra.ins.sync_info = mybir.SyncInfo(
                on_wait=[w],
                on_update=list(esi.on_update or []) if esi else [],
            )
    nc.all_engine_barrier()
    assert self.sems is not None
    popped = nc._tile_sem_poison_stack.pop()
    assert popped is self._sem_poison
    nc.clear_and_free_semaphores(list(self.sems.allocated().values()))
    nc.all_engine_barrier()


def _apply_tilefix():
    tile.TileContext._commit_instruction = _commit_instruction
    tile.TileContext._drain_and_barrier = _drain_and_barrier


_apply_tilefix()

# ---------------------------------------------------------------------------
# Problem constants (hardcoded)
# ---------------------------------------------------------------------------
B, S, E, H = 2, 2048, 1024, 16
HC, D = 4, 64              # heads per core, head dim
NCORES = 8
NE = E // 128              # 8  e-chunks
NW = S // 512              # 4  q/k windows
NK = S // 128              # 16 k-chunks
NM = S // 128              # 16 m-chunks

F32 = mybir.dt.float32
FP16 = mybir.dt.float16


def build(ptbufs=18):
    nc = bass.Bass()
    xqT = nc.dram_tensor("xqT", [E, S], FP16, kind="ExternalInput")
    xkT = nc.dram_tensor("xkT", [E, S], FP16, kind="ExternalInput")
    xvT = nc.dram_tensor("xvT", [E, S], FP16, kind="ExternalInput")
    wqT = nc.dram_tensor("wqT", [E, 256], FP16, kind="ExternalInput")
    wkT = nc.dram_tensor("wkT", [E, 256], FP16, kind="ExternalInput")
    wvT = nc.dram_tensor("wvT", [E, 256], FP16, kind="ExternalInput")
    woT = nc.dram_tensor("woT", [256, E], FP16, kind="ExternalInput")
    out = nc.dram_tensor("out", [S, E], F32, kind="ExternalOutput")
    out16 = nc.dram_tensor("out16", [512, E], FP16, kind="ExternalOutput")
    identT = nc.dram_tensor("identT", [128, 128], FP16, kind="ExternalInput")

    with tile.TileContext(nc) as tc, ExitStack() as ctx:
        consts = ctx.enter_context(tc.tile_pool(name="consts", bufs=1))
        wpool = ctx.enter_context(tc.tile_pool(name="w", bufs=1))
        actpool = ctx.enter_context(tc.tile_pool(name="acts", bufs=1))
        xkpool = ctx.enter_context(tc.tile_pool(name="xk", bufs=3))
        xqpool = ctx.enter_context(tc.tile_pool(name="xq", bufs=3))
        xvpool = ctx.enter_context(tc.tile_pool(name="xv", bufs=3))
        ptpool = ctx.enter_context(tc.tile_pool(name="pT", bufs=ptbufs))
        onpool = ctx.enter_context(tc.tile_pool(name="on", bufs=4))
        rpool = ctx.enter_context(tc.tile_pool(name="recip", bufs=8))
        opool = ctx.enter_context(tc.tile_pool(name="outstage", bufs=2))
        ohpool = ctx.enter_context(tc.tile_pool(name="outhalf", bufs=8))
        psS = ctx.enter_context(tc.tile_pool(name="psS", bufs=2, space="PSUM"))
        psOV = ctx.enter_context(tc.tile_pool(name="psOV", bufs=1, space="PSUM"))
        psX = ctx.enter_context(tc.tile_pool(name="psX", bufs=2, space="PSUM"))

        # preload the exp table before the hot loop
        dummy = consts.tile([1, 8], F32)
        nc.vector.memset(dummy[:], 0.0)
        nc.scalar.activation(dummy[:], dummy[:], mybir.ActivationFunctionType.Exp)

        wq_sb = wpool.tile([128, NE, 256], FP16)
        wk_sb = wpool.tile([128, NE, 256], FP16)
        wv_sb = wpool.tile([128, NE, 256], FP16)
        wo_sb = wpool.tile([128, 2, E], FP16)

        qT_sb = actpool.tile([128, 2, S], FP16)        # [(2 heads x d), pair, s]
        kT_sb = actpool.tile([128, 2, S], FP16)
        v_sb = actpool.tile([128, NK, HC, 65], FP16)   # [s%128, k, h, V_h|ones]
        oT_sb = actpool.tile([128, 2, S], FP16, name="oT")  # [(h2 d), pair, s]

        nc.vector.memset(v_sb[:, :, :, 64:65], 1.0)

        # ---- DMA emission order on the SP queue (arrival order == need) ---
        def colblock(x, j):
            return x[:, j * 512:(j + 1) * 512].rearrange(
                "(ec p) s -> p ec s", p=128)

        def halfblock(x, j, h):
            return x[h * 512:(h + 1) * 512,
                     j * 512:(j + 1) * 512].rearrange(
                "(ec p) s -> p ec s", p=128)

        xq_blks = {}
        xk_blks = {}
        xv_blks = {}

        def load_x(pool, src, blks, j, tag, split=False):
            t = pool.tile([128, NE, 512], FP16, tag=tag, name=f"{tag}{j}")
            if split:
                # two half-e DMAs so the projection can chase the first half
                nc.sync.dma_start(t[:, 0:4, :], halfblock(src, j, 0))
                nc.sync.dma_start(t[:, 4:8, :], halfblock(src, j, 1))
            else:
                nc.sync.dma_start(t[:], colblock(src, j))
            blks[j] = t

        # arrival order == need order (single serialized DMA device)
        nc.sync.dma_start(wk_sb[:], wkT.rearrange("(ec p) n -> p ec n", p=128))
        load_x(xkpool, xkT, xk_blks, 0, "xkb", split=True)
        nc.sync.dma_start(wq_sb[:], wqT.rearrange("(ec p) n -> p ec n", p=128))
        load_x(xqpool, xqT, xq_blks, 0, "xqb", split=True)
        for j in range(1, NW):
            load_x(xkpool, xkT, xk_blks, j, "xkb", split=True)
        nc.sync.dma_start(wv_sb[:], wvT.rearrange("(ec p) n -> p ec n", p=128))
        load_x(xvpool, xvT, xv_blks, 0, "xvb")
        load_x(xvpool, xvT, xv_blks, 1, "xvb")
        load_x(xqpool, xqT, xq_blks, 1, "xqb")
        load_x(xvpool, xvT, xv_blks, 2, "xvb")
        load_x(xvpool, xvT, xv_blks, 3, "xvb")
        load_x(xqpool, xqT, xq_blks, 2, "xqb")
        load_x(xqpool, xqT, xq_blks, 3, "xqb")
        nc.sync.dma_start(wo_sb[:], woT.rearrange("(j p) e -> p j e", p=128))
        id_sb = wpool.tile([128, 128], FP16)
        nc.sync.dma_start(id_sb[:], identT[:, :])

        # ---- building blocks -------------------------------------------
        def wproj(w_sb, blk, dst, win):
            """project one 512-col window of x into dst[:, :, win] (2 psX)."""
            ws = slice(win * 512, (win + 1) * 512)
            for nch in range(2):
                ps = psX.tile([128, 512], F32, tag="px", name=f"pj{win}_{nch}")
                for e in range(NE):
                    nc.tensor.matmul(
                        ps[:],
                        w_sb[:, e, nch * 128:(nch + 1) * 128],
                        blk[:, e, :],
                        start=(e == 0), stop=(e == NE - 1))
                nc.vector.tensor_copy(dst[:, nch, ws], ps[:])

        def vproj(m):
            blk = xv_blks[m // 4]
            ps = psX.tile([128, 512], F32, tag="px", name=f"vp{m}")
            for e in range(NE):
                nc.tensor.matmul(
                    ps[:, 0:256],
                    blk[:, e, (m % 4) * 128:(m % 4 + 1) * 128],
                    wv_sb[:, e, :],
                    start=(e == 0), stop=(e == NE - 1))
            nc.vector.tensor_copy(
                v_sb[:, m, :, 0:64],
                ps[:, 0:256].rearrange("p (h c) -> p h c", h=HC))

        def ov_group(ovts, pair, pT, kc):
            """8 transposed-PV matmuls for one k-chunk; sub-bank psum accum."""
            for t_i, (ovt, qlocs) in enumerate(ovts):
                for si, (h2, ql) in enumerate(
                        [(h, q) for h in range(2) for q in qlocs]):
                    nc.tensor.matmul(
                        ovt[:, ql % 2, h2, :],
                        pT[:, h2 * 512 + ql * 128: h2 * 512 + (ql + 1) * 128],
                        v_sb[:, kc, 2 * pair + h2, :],
                        start=(kc == 0 and si == 0),
                        stop=(kc == NK - 1 and si == 3),
                        skip_group_check=True)

        def finalize(ovts, w, pair):
            """normalize + crossbar-transpose one (window, pair)."""
            for ovt, qlocs in ovts:
                for ql in qlocs:
                    o_n = onpool.tile([128, 128], FP16, tag="on")
                    for h2 in range(2):
                        rt = rpool.tile([128, 1], F32, tag="rt")
                        nc.vector.reciprocal(rt[:], ovt[:, ql % 2, h2, 64:65])
                        nc.vector.tensor_scalar_mul(
                            o_n[:, h2 * 64:(h2 + 1) * 64],
                            ovt[:, ql % 2, h2, 0:64],
                            rt[:])
                    qs = slice(w * 512 + ql * 128, w * 512 + (ql + 1) * 128)
                    nc.sync.dma_start_transpose(oT_sb[:, pair, qs], o_n[:])

        def outproj_half(m, j, stage):
            ps = psX.tile([128, 512], F32, tag="px", name=f"op{m}_{j}")
            for jp in range(2):
                nc.tensor.matmul(
                    ps[:],
                    oT_sb[:, jp, m * 128:(m + 1) * 128],
                    wo_sb[:, jp, j * 512:(j + 1) * 512],
                    start=(jp == 0), stop=(jp == 1))
            nc.vector.tensor_copy(stage[:, j * 512:(j + 1) * 512], ps[:])
            if j == 1:
                nc.gpsimd.dma_start(out[m * 128:(m + 1) * 128, :], stage[:])

        # ---- global-slot schedule --------------------------------------
        # slot g = p*16 + kc carries scores(p,kc)+exp; OV work is lagged
        # OVLAG slots behind the exp stream (rolling across p boundaries).
        OVLAG = 10
        TAILOV = 8            # OV groups left for the post-stream tail
        from collections import defaultdict
        extras_pre = defaultdict(list)    # g -> thunks (feeders: proj work)
        extras_post = defaultdict(list)   # g -> thunks (drains: outproj)

        def sched_wproj(g, w_sb, blks, dst, win, nch=None):
            for n in ((0, 1) if nch is None else (nch,)):
                extras_pre[g].append(
                    lambda n=n, win=win: wproj1(w_sb, blks[win], dst, win, n))

        def wproj1(w_sb, blk, dst, win, nch, use_act=False):
            ws = slice(win * 512, (win + 1) * 512)
            ps = psX.tile([128, 512], F32, tag="px", name=f"pj{win}_{nch}")
            for e in range(NE):
                nc.tensor.matmul(
                    ps[:],
                    w_sb[:, e, nch * 128:(nch + 1) * 128],
                    blk[:, e, :],
                    start=(e == 0), stop=(e == NE - 1))
            if use_act:
                nc.scalar.copy(dst[:, nch, ws], ps[:])
            else:
                nc.vector.tensor_copy(dst[:, nch, ws], ps[:])

        # K windows 1-3 early in p0 (chasing the xk block DMAs)
        sched_wproj(2, wk_sb, xk_blks, kT_sb, 1, nch=0)
        sched_wproj(3, wk_sb, xk_blks, kT_sb, 1, nch=1)
        sched_wproj(6, wk_sb, xk_blks, kT_sb, 2, nch=0)
        sched_wproj(7, wk_sb, xk_blks, kT_sb, 2, nch=1)
        sched_wproj(10, wk_sb, xk_blks, kT_sb, 3, nch=0)
        sched_wproj(11, wk_sb, xk_blks, kT_sb, 3, nch=1)
        # V tiles 2 slots ahead of their OV consumer (max arrival slack)
        for m in range(NM):
            extras_pre[m + 10].append(lambda m=m: vproj(m))
        # remaining Q windows, off the critical path, in slack windows
        sched_wproj(18, wq_sb, xq_blks, qT_sb, 1, nch=0)
        sched_wproj(20, wq_sb, xq_blks, qT_sb, 1, nch=1)
        sched_wproj(40, wq_sb, xq_blks, qT_sb, 2, nch=0)
        sched_wproj(56, wq_sb, xq_blks, qT_sb, 2, nch=1)
        sched_wproj(72, wq_sb, xq_blks, qT_sb, 3, nch=0)
        sched_wproj(88, wq_sb, xq_blks, qT_sb, 3, nch=1)

        # outproj window w' after both its finalizes: units u0..3 at
        # p(2w'+2) slots 12..15, u4..7 at p(2w'+3) slots 1,3,5,7
        stages = {}

        def outproj_unit(opw, u):
            m = opw * 4 + u // 2
            if u % 2 == 0:
                stages[m] = opool.tile([128, E], F32, tag="ost", name=f"st{m}")
            outproj_half(m, u % 2, stages[m])

        for opw in range(3):
            for u in range(4):
                extras_post[(2 * opw + 2) * 16 + 12 + u].append(
                    lambda opw=opw, u=u: outproj_unit(opw, u))
            for u in range(4, 8):
                extras_post[(2 * opw + 3) * 16 + 2 * (u - 4) + 1].append(
                    lambda opw=opw, u=u: outproj_unit(opw, u))

        # OV emission: per-pair lag shrinks toward the end so the last
        # pair's PV work lands almost entirely in-stream; transitions
        # auto-bunch to 2 groups/slot.  finalize(pp) sits between pairs.
        LAGS = [10, 10, 10, 10, 8, 6, 4, 2]
        ovmap = defaultdict(list)     # g -> ordered ops: ("ov", n) | ("fin", pp)
        tail_ops = []
        gcur = 0
        for pp in range(8):
            for kc in range(16):
                n = pp * 16 + kc
                g = max(pp * 16 + kc + LAGS[pp], gcur)
                while g <= 127 and sum(
                        1 for o in ovmap[g] if o[0] == "ov") >= 2:
                    g += 1
                if g > 127:
                    tail_ops.append(("ov", n))
                else:
                    ovmap[g].append(("ov", n))
                    gcur = g
            if pp < 7:
                fg = gcur + 1 if not tail_ops else None
                if fg is not None and fg <= 127:
                    ovmap[fg].insert(0, ("fin", pp))
                    gcur = fg   # next pair's groups must follow the fin
                else:
                    tail_ops.append(("fin", pp))
        n_total = 128 - len([o for o in tail_ops if o[0] == "ov"])

        # ---- warmup: anchor the PE p-state ramp while DMAs stream ------
        wu_a = consts.tile([128, 128], FP16)
        wu_b = consts.tile([128, 64], FP16)
        nc.vector.memset(wu_a[:], 0.0)
        nc.vector.memset(wu_b[:], 0.0)
        for i in range(3):
            ps = psX.tile([128, 512], F32, tag="px", name=f"wu{i}")
            nc.tensor.matmul(ps[:, 0:64], wu_a[:], wu_b[:], start=True, stop=True)

        wproj1(wk_sb, xk_blks[0], kT_sb, 0, 0, use_act=True)
        wproj1(wk_sb, xk_blks[0], kT_sb, 0, 1, use_act=True)
        wproj1(wq_sb, xq_blks[0], qT_sb, 0, 0, use_act=True)
        wproj1(wq_sb, xq_blks[0], qT_sb, 0, 1, use_act=True)

        pts = {}
        ovts = None
        for g in range(128):
            p, kc = divmod(g, 16)
            w, pair = divmod(p, 2)
            qs = slice(w * 512, (w + 1) * 512)
            ks = slice(kc * 128, (kc + 1) * 128)

            ps_s = psS.tile([128, 1024], F32)
            nc.tensor.matmul(ps_s[:, 0:512],
                             kT_sb[0:64, pair, ks],
                             qT_sb[0:64, pair, qs],
                             start=True, stop=True)
            nc.tensor.matmul(ps_s[:, 512:1024],
                             kT_sb[64:128, pair, ks],
                             qT_sb[64:128, pair, qs],
                             start=True, stop=True)
            pT = ptpool.tile([128, 1024], FP16, tag="pT")
            nc.scalar.activation(pT[:], ps_s[:],
                                 mybir.ActivationFunctionType.Exp,
                                 scale=0.125)
            pts[g] = pT

            for thunk in extras_pre.get(g, ()):
                thunk()
            for op, val in ovmap.get(g, ()):
                if op == "fin":
                    finalize(ovts, val // 2, val % 2)
                    continue
                pp, pkc = divmod(val, 16)
                if pkc == 0:
                    ovA = psOV.tile([128, 2, 2, 65], F32, tag="ovA",
                                    name=f"ovA{pp}")
                    ovB = psOV.tile([128, 2, 2, 65], F32, tag="ovB",
                                    name=f"ovB{pp}")
                    ovts = ((ovA, (0, 1)), (ovB, (2, 3)))
                ov_group(ovts, pp % 2, pts.pop(val), pkc)
            for thunk in extras_post.get(g, ()):
                thunk()

        # ---- tail: leftover OV/fin ops, then phase-ordered finalize +
        # outproj; normalize split across DVE/ACT, scores PSUM banks reused
        # for extra outproj accumulators so the 8 halves pipeline deeply.
        for op, val in tail_ops:
            if op == "fin":
                finalize(ovts, val // 2, val % 2)
            else:
                pp, pkc = divmod(val, 16)
                if pkc == 0:
                    ovA = psOV.tile([128, 2, 2, 65], F32, tag="ovA",
                                    name=f"ovA{pp}")
                    ovB = psOV.tile([128, 2, 2, 65], F32, tag="ovB",
                                    name=f"ovB{pp}")
                    ovts = ((ovA, (0, 1)), (ovB, (2, 3)))
                ov_group(ovts, pp % 2, pts.pop(val), pkc)
        psA = psS.tile([128, 1024], F32, tag="ps_s", name="psA")
        psB = psS.tile([128, 1024], F32, tag="ps_s", name="psB")
        for ql in range(4):
            ovt = ovts[0] if ql < 2 else ovts[1]
            o_n = onpool.tile([128, 128], FP16, tag="on")
            for h2 in range(2):
                rt = rpool.tile([128, 1], F32, tag="rt")
                nc.vector.reciprocal(rt[:], ovt[0][:, ql % 2, h2, 64:65])
                if h2 == 0:
                    nc.vector.tensor_scalar_mul(
                        o_n[:, 0:64], ovt[0][:, ql % 2, 0, 0:64], rt[:])
                else:
                    nc.scalar.activation(
                        o_n[:, 64:128], ovt[0][:, ql % 2, 1, 0:64],
                        mybir.ActivationFunctionType.Copy, scale=rt[:])
            # PE transpose (53ns) beats the crossbar DMA round-trip here;
            # scratch lives in the dead scores tiles' second zero-region
            tsrc = (psA, psB)[ql // 2]
            off = 768 + (ql % 2) * 64
            tps = tsrc[:, off:off + 64].bitcast(FP16)
            nc.tensor.transpose(tps, o_n[:], id_sb[:])
            qs = slice(3 * 512 + ql * 128, 3 * 512 + (ql + 1) * 128)
            nc.vector.tensor_copy(oT_sb[:, 1, qs], tps)
        for ql in range(4):
            m = 12 + ql
            for j in range(2):
                if j == 0:
                    ps = (psA, psB)[ql % 2][:, 0:512]
                else:
                    pst = psX.tile([128, 512], F32, tag="px",
                                   name=f"tp{m}")
                    ps = pst[:]
                for jp in range(2):
                    nc.tensor.matmul(
                        ps,
                        oT_sb[:, jp, m * 128:(m + 1) * 128],
                        wo_sb[:, jp, j * 512:(j + 1) * 512],
                        start=(jp == 0), stop=(jp == 1))
                half = ohpool.tile([128, 512], FP16, tag="osth",
                                   name=f"sth{m}_{j}")
                if j == 0:
                    nc.vector.tensor_copy(half[:], ps)
                else:
                    nc.scalar.copy(half[:], ps)
                nc.sync.dma_start(
                    out16[(m - 12) * 128:(m - 11) * 128,
                          j * 512:(j + 1) * 512],
                    half[:])

    return nc


_NC_CACHE = {}


def _get_nc():
    if "nc" not in _NC_CACHE:
        _NC_CACHE["nc"] = build()
    return _NC_CACHE["nc"]


def _shard_inputs(query, key, value, Wq, Wk, Wv, Wo):
    """Host-side sharding + layout prep: core c = (batch c//4, head-group c%4)."""
    f16 = np.float16
    xT = []
    for b in range(B):
        xT.append((
            np.ascontiguousarray(query[b].T).astype(f16),
            np.ascontiguousarray(key[b].T).astype(f16),
            np.ascontiguousarray(value[b].T).astype(f16),
        ))
    ident = np.eye(128, dtype=f16)
    wT = []
    for g in range(4):
        gc = slice(g * 256, (g + 1) * 256)
        wT.append((
            np.ascontiguousarray(Wq[gc].T).astype(f16),
            np.ascontiguousarray(Wk[gc].T).astype(f16),
            np.ascontiguousarray(Wv[gc].T).astype(f16),
            np.ascontiguousarray(Wo[:, gc].T).astype(f16),
        ))
    in_maps = []
    for c in range(NCORES):
        b, g = c // 4, c % 4
        qT, kT, vT = xT[b]
        wq, wk, wv, wo = wT[g]
        in_maps.append({
            "xqT": qT, "xkT": kT, "xvT": vT,
            "wqT": wq, "wkT": wk, "wvT": wv, "woT": wo,
            "identT": ident,
        })
    return in_maps


def kernel(query, key, value, Wq, Wk, Wv, Wo):
    query = np.asarray(query, dtype=np.float32)
    key = np.asarray(key, dtype=np.float32)
    value = np.asarray(value, dtype=np.float32)
    Wq = np.asarray(Wq, dtype=np.float32)
    Wk = np.asarray(Wk, dtype=np.float32)
    Wv = np.asarray(Wv, dtype=np.float32)
    Wo = np.asarray(Wo, dtype=np.float32)

    nc = _get_nc()
    in_maps = _shard_inputs(query, key, value, Wq, Wk, Wv, Wo)
    res = run_bass_kernel_spmd(nc, in_maps, core_ids=list(range(NCORES)))

    out = np.zeros((B, S, E), dtype=np.float32)
    for c in range(NCORES):
        out[c // 4][0:1536] += res.results[c]["out"][0:1536]
        out[c // 4][1536:2048] += res.results[c]["out16"].astype(np.float32)
    return out
